# revision 15
# baseline (speedup 1.0000x reference)
"""Chamfer point-cloud completion loss on 8 Trainium2 NeuronCores.

Strategy: data-parallel over (batch, row-half). Core c handles batch c//2,
row-half c%2 of both the coarse (1024 pts) and fine (8192 pts) clouds vs the
full gt cloud (8192 pts) of that batch.

Squared L2 distances via a K=5 "lift" matmul on the PE:
  lhsT = [x0, x1, x2, |x|^2, 1]      (K=5, M=128 x-points)
  rhs  = [-2y0, -2y1, -2y2, 1, |y|^2] (K=5, N=512 y-points)
  out[m, n] = |x_m|^2 + |y_n|^2 - 2 x_m.y_n = d(m, n)   (PSUM, f32)

Row-mins (over gt) via DVE tensor_reduce per PSUM group; col-mins (over pred)
via in-place DVE/GpSimd tensor_tensor min into an SBUF accumulator
colacc[128, 8192] (partition p holds min over rows r = p mod 128 of the
core's shard). Host finishes: partition-min of colacc, pair-min across the
two cores of each batch, float64 means, scale by params.
"""

import os
import sys

import numpy as np

_TRN_REPO = "/opt/trn_rl_repo"
if _TRN_REPO not in sys.path:
    sys.path.insert(0, _TRN_REPO)

B = 4
N_COARSE = 1024
N_FINE = 8192
N_GT = 8192
N_CORES = 8

ROWS_COARSE = N_COARSE // 2   # 512 rows per core
ROWS_FINE = N_FINE // 2       # 4096 rows per core
ROWS_TOTAL = ROWS_COARSE + ROWS_FINE  # 4608
RB_COARSE = ROWS_COARSE // 128  # 4 row-blocks (first)
RB_FINE = ROWS_FINE // 128      # 32 row-blocks
RB_TOTAL = RB_COARSE + RB_FINE  # 36

GROUP_W = 2048                # PSUM group width (4 banks)
N_GROUPS = N_GT // GROUP_W    # 4 groups per row-block
MM_W = 512                    # single matmul moving width
MM_PER_GROUP = GROUP_W // MM_W

LAST_EXEC_TIME_NS = None

_CACHED = {}


def _build_nc():
    import concourse.bass as bass
    import concourse.tile as tile
    from concourse import mybir
    from concourse.bacc import Bacc

    f32 = mybir.dt.float32
    f32r = mybir.dt.float32r
    AX = mybir.AxisListType
    OP = mybir.AluOpType

    nc = Bacc()

    xy_lift_d = nc.dram_tensor(
        "xy_lift", [5, ROWS_TOTAL + N_GT], f32, kind="ExternalInput"
    )
    out_rowmin_d = nc.dram_tensor("out_rowmin", [128, RB_TOTAL], f32, kind="ExternalOutput")
    out_colacc_c_d = nc.dram_tensor("out_colacc_coarse", [128, N_GT], f32, kind="ExternalOutput")
    out_colacc_f_d = nc.dram_tensor("out_colacc_fine", [128, N_GT], f32, kind="ExternalOutput")

    BIG = 3.0e38

    with tile.TileContext(nc) as tc:
        with (
            tc.tile_pool(name="singles", bufs=1) as singles,
            tc.tile_pool(name="work", bufs=3) as work,
            tc.tile_pool(name="psum", bufs=2, space="PSUM") as psum_pool,
        ):
            xy_lift = singles.tile([5, ROWS_TOTAL + N_GT], f32)
            nc.sync.dma_start(out=xy_lift[:], in_=xy_lift_d[:])
            x_lift = xy_lift[:, :ROWS_TOTAL]
            y_lift = xy_lift[:, ROWS_TOTAL:]

            colacc_c = singles.tile([128, N_GT], f32)
            colacc_f = singles.tile([128, N_GT], f32)
            rowmin_all = singles.tile([128, RB_TOTAL], f32)

            nc.vector.memset(colacc_c[:], BIG)
            nc.vector.memset(colacc_f[:], BIG)

            for rb in range(RB_TOTAL):
                colacc = colacc_c if rb < RB_COARSE else colacc_f
                rowmin_tmp = work.tile([128, N_GROUPS], f32, name="rowmin_tmp")
                for g in range(N_GROUPS):
                    pg = psum_pool.tile([128, GROUP_W], f32, name="pg")
                    for k in range(MM_PER_GROUP):
                        c0 = g * GROUP_W + k * MM_W
                        nc.tensor.matmul(
                            pg[:, k * MM_W:(k + 1) * MM_W],
                            x_lift[:, rb * 128:(rb + 1) * 128],
                            y_lift[:, c0:c0 + MM_W],
                        )
                    nc.vector.tensor_reduce(
                        out=rowmin_tmp[:, g:g + 1], in_=pg[:], axis=AX.X, op=OP.min,
                    )
                    acc = colacc[:, g * GROUP_W:(g + 1) * GROUP_W]
                    nc.vector.tensor_tensor(out=acc, in0=pg[:], in1=acc, op=OP.min)
                nc.vector.tensor_reduce(
                    out=rowmin_all[:, rb:rb + 1], in_=rowmin_tmp[:], axis=AX.X, op=OP.min,
                )

            nc.sync.dma_start(out=out_rowmin_d[:], in_=rowmin_all[:])
            nc.sync.dma_start(out=out_colacc_c_d[:], in_=colacc_c[:])
            nc.sync.dma_start(out=out_colacc_f_d[:], in_=colacc_f[:])

    nc.finalize()
    return nc


def _lift_inputs(coarse_pc, fine_pc, gt_pc):
    """Per-core lifted input arrays."""
    in_maps = []
    for c in range(N_CORES):
        b, h = divmod(c, 2)
        C = coarse_pc[b, h * ROWS_COARSE:(h + 1) * ROWS_COARSE]  # [512, 3]
        F = fine_pc[b, h * ROWS_FINE:(h + 1) * ROWS_FINE]        # [4096, 3]
        X = np.concatenate([C, F], axis=0).astype(np.float32)    # [4608, 3]
        Y = gt_pc[b].astype(np.float32)                          # [8192, 3]
        xy_lift = np.empty((5, ROWS_TOTAL + N_GT), dtype=np.float32)
        xy_lift[0:3, :ROWS_TOTAL] = X.T
        xy_lift[3, :ROWS_TOTAL] = (X * X).sum(axis=1)
        xy_lift[4, :ROWS_TOTAL] = 1.0
        xy_lift[0:3, ROWS_TOTAL:] = -2.0 * Y.T
        xy_lift[3, ROWS_TOTAL:] = 1.0
        xy_lift[4, ROWS_TOTAL:] = (Y * Y).sum(axis=1)

        in_maps.append({"xy_lift": xy_lift})
    return in_maps


def kernel(coarse_pc, fine_pc, gt_pc, param_coarse, param_fine):
    global LAST_EXEC_TIME_NS
    from concourse.bass_utils import run_bass_kernel_spmd

    coarse_pc = np.asarray(coarse_pc, dtype=np.float32)
    fine_pc = np.asarray(fine_pc, dtype=np.float32)
    gt_pc = np.asarray(gt_pc, dtype=np.float32)

    if "nc" not in _CACHED:
        _CACHED["nc"] = _build_nc()
    nc = _CACHED["nc"]

    in_maps = _lift_inputs(coarse_pc, fine_pc, gt_pc)
    trace = bool(os.environ.get("CHAMFER_TRACE"))
    res = run_bass_kernel_spmd(nc, in_maps, core_ids=list(range(N_CORES)), trace=trace)
    LAST_EXEC_TIME_NS = res.exec_time_ns
    results = res.results

    rowmin_c_sum = 0.0
    rowmin_f_sum = 0.0
    col_c_sum = 0.0
    col_f_sum = 0.0
    for b in range(B):
        r0 = results[2 * b]
        r1 = results[2 * b + 1]
        for r in (r0, r1):
            rm = r["out_rowmin"]
            rowmin_c_sum += rm[:, :RB_COARSE].sum(dtype=np.float64)
            rowmin_f_sum += rm[:, RB_COARSE:].sum(dtype=np.float64)
        col_c = np.minimum(r0["out_colacc_coarse"], r1["out_colacc_coarse"]).min(axis=0)
        col_f = np.minimum(r0["out_colacc_fine"], r1["out_colacc_fine"]).min(axis=0)
        col_c_sum += col_c.sum(dtype=np.float64)
        col_f_sum += col_f.sum(dtype=np.float64)

    loss_coarse = (rowmin_c_sum / (B * N_COARSE) + col_c_sum / (B * N_GT)) * float(param_coarse)
    loss_fine = (rowmin_f_sum / (B * N_FINE) + col_f_sum / (B * N_GT)) * float(param_fine)
    return np.array([loss_coarse, loss_fine], dtype=np.float32)


# revision 19
# speedup vs baseline: 1.5130x; 1.5130x over previous
"""Chamfer point-cloud completion loss on 8 Trainium2 NeuronCores.

Strategy: data-parallel over (batch, row-half). Core c handles batch c//2,
row-half c%2 of both the coarse (1024 pts) and fine (8192 pts) clouds vs the
full gt cloud (8192 pts) of that batch.

Squared L2 distances via a "lift" matmul on the PE:
  lift_x = [x0, x1, x2, |x|^2, 1]      (5 rows, M=128 x-points)
  lift_y = [-2y0, -2y1, -2y2, 1, |y|^2] (5 rows, N=512 y-points)
  d(m, n) = sum_k lift_x[k, m] * lift_y[k, n]
To get fp32-class precision at bf16 matmul speed (1 cycle/row vs 4 for
fp32), each lift row v is decomposed v = h + m + l (3x bf16) and the six
product terms hh', hm', mh', hl', lh', mm' are stacked into ONE K=30
bf16 matmul (cost is independent of K); dropped terms are O(2^-26).

Row-mins (over gt) via DVE tensor_reduce per PSUM group; col-mins (over pred)
via in-place DVE/GpSimd tensor_tensor min into an SBUF accumulator
colacc[128, 8192] (partition p holds min over rows r = p mod 128 of the
core's shard). Host finishes: partition-min of colacc, pair-min across the
two cores of each batch, float64 means, scale by params.
"""

import os
import sys

import numpy as np

_TRN_REPO = "/opt/trn_rl_repo"
if _TRN_REPO not in sys.path:
    sys.path.insert(0, _TRN_REPO)

B = 4
N_COARSE = 1024
N_FINE = 8192
N_GT = 8192
N_CORES = 8

ROWS_COARSE = N_COARSE // 2   # 512 rows per core
ROWS_FINE = N_FINE // 2       # 4096 rows per core
ROWS_TOTAL = ROWS_COARSE + ROWS_FINE  # 4608
RB_COARSE = ROWS_COARSE // 128  # 4 row-blocks (first)
RB_FINE = ROWS_FINE // 128      # 32 row-blocks
RB_TOTAL = RB_COARSE + RB_FINE  # 36

GROUP_W = 2048                # PSUM group width (4 banks)
N_GROUPS = N_GT // GROUP_W    # 4 groups per row-block
MM_W = 512                    # single matmul moving width
MM_PER_GROUP = GROUP_W // MM_W

LAST_EXEC_TIME_NS = None

_CACHED = {}


def _build_nc():
    import concourse.bass as bass
    import concourse.tile as tile
    from concourse import mybir
    from concourse.bacc import Bacc

    f32 = mybir.dt.float32
    bf16 = mybir.dt.bfloat16
    AX = mybir.AxisListType
    OP = mybir.AluOpType

    nc = Bacc()

    xy_lift_d = nc.dram_tensor(
        "xy_lift", [30, ROWS_TOTAL + N_GT], bf16, kind="ExternalInput"
    )
    out_rowmin_d = nc.dram_tensor("out_rowmin", [128, RB_TOTAL], f32, kind="ExternalOutput")
    out_colacc_c_d = nc.dram_tensor("out_colacc_coarse", [128, N_GT], f32, kind="ExternalOutput")
    out_colacc_f_d = nc.dram_tensor("out_colacc_fine", [128, N_GT], f32, kind="ExternalOutput")

    BIG = 3.0e38

    with tile.TileContext(nc) as tc:
        with (
            tc.tile_pool(name="singles", bufs=1) as singles,
            tc.tile_pool(name="work", bufs=3) as work,
            tc.tile_pool(name="psum", bufs=2, space="PSUM") as psum_pool,
        ):
            xy_lift = singles.tile([30, ROWS_TOTAL + N_GT], bf16)
            nc.sync.dma_start(out=xy_lift[:], in_=xy_lift_d[:])
            x_lift = xy_lift[:, :ROWS_TOTAL]
            y_lift = xy_lift[:, ROWS_TOTAL:]

            colacc_c = singles.tile([128, N_GT], f32)
            colacc_f = singles.tile([128, N_GT], f32)
            rowmin_all = singles.tile([128, RB_TOTAL], f32)

            nc.vector.memset(colacc_c[:], BIG)
            nc.vector.memset(colacc_f[:], BIG)

            for rb in range(RB_TOTAL):
                colacc = colacc_c if rb < RB_COARSE else colacc_f
                rowmin_tmp = work.tile([128, N_GROUPS], f32, name="rowmin_tmp")
                for g in range(N_GROUPS):
                    pg = psum_pool.tile([128, GROUP_W], f32, name="pg")
                    for k in range(MM_PER_GROUP):
                        c0 = g * GROUP_W + k * MM_W
                        nc.tensor.matmul(
                            pg[:, k * MM_W:(k + 1) * MM_W],
                            x_lift[:, rb * 128:(rb + 1) * 128],
                            y_lift[:, c0:c0 + MM_W],
                        )
                    nc.vector.tensor_reduce(
                        out=rowmin_tmp[:, g:g + 1], in_=pg[:], axis=AX.X, op=OP.min,
                    )
                    acc = colacc[:, g * GROUP_W:(g + 1) * GROUP_W]
                    nc.vector.tensor_tensor(out=acc, in0=pg[:], in1=acc, op=OP.min)
                nc.vector.tensor_reduce(
                    out=rowmin_all[:, rb:rb + 1], in_=rowmin_tmp[:], axis=AX.X, op=OP.min,
                )

            nc.sync.dma_start(out=out_rowmin_d[:], in_=rowmin_all[:])
            nc.sync.dma_start(out=out_colacc_c_d[:], in_=colacc_c[:])
            nc.sync.dma_start(out=out_colacc_f_d[:], in_=colacc_f[:])

    nc.finalize()
    return nc


def _bf16_split3(v):
    """v (f32/f64) -> (h, m, l) bf16 arrays with h+m+l ~= v to ~2^-26."""
    import ml_dtypes

    bf = ml_dtypes.bfloat16
    v = v.astype(np.float64)
    h = v.astype(bf)
    r = v - h.astype(np.float64)
    m = r.astype(bf)
    l = (r - m.astype(np.float64)).astype(bf)
    return h, m, l


def _lift_inputs(coarse_pc, fine_pc, gt_pc):
    """Per-core lifted input arrays (bf16 triple-decomposed, K=30)."""
    import ml_dtypes

    bf = ml_dtypes.bfloat16
    in_maps = []
    for c in range(N_CORES):
        b, h = divmod(c, 2)
        C = coarse_pc[b, h * ROWS_COARSE:(h + 1) * ROWS_COARSE]  # [512, 3]
        F = fine_pc[b, h * ROWS_FINE:(h + 1) * ROWS_FINE]        # [4096, 3]
        X = np.concatenate([C, F], axis=0).astype(np.float64)    # [4608, 3]
        Y = gt_pc[b].astype(np.float64)                          # [8192, 3]

        lift_x = np.empty((5, ROWS_TOTAL), dtype=np.float64)
        lift_x[0:3] = X.T
        lift_x[3] = (X * X).sum(axis=1)
        lift_x[4] = 1.0
        lift_y = np.empty((5, N_GT), dtype=np.float64)
        lift_y[0:3] = -2.0 * Y.T
        lift_y[3] = 1.0
        lift_y[4] = (Y * Y).sum(axis=1)

        xh, xm, xl = _bf16_split3(lift_x)
        yh, ym, yl = _bf16_split3(lift_y)

        # sum over the six stacked blocks = hh' + hm' + mh' + hl' + lh' + mm'
        x_blocks = (xh, xh, xm, xh, xl, xm)
        y_blocks = (yh, ym, yh, yl, yh, ym)
        xy_lift = np.empty((30, ROWS_TOTAL + N_GT), dtype=bf)
        for i in range(6):
            xy_lift[5 * i:5 * i + 5, :ROWS_TOTAL] = x_blocks[i]
            xy_lift[5 * i:5 * i + 5, ROWS_TOTAL:] = y_blocks[i]

        in_maps.append({"xy_lift": xy_lift})
    return in_maps


def kernel(coarse_pc, fine_pc, gt_pc, param_coarse, param_fine):
    global LAST_EXEC_TIME_NS
    from concourse.bass_utils import run_bass_kernel_spmd

    coarse_pc = np.asarray(coarse_pc, dtype=np.float32)
    fine_pc = np.asarray(fine_pc, dtype=np.float32)
    gt_pc = np.asarray(gt_pc, dtype=np.float32)

    if "nc" not in _CACHED:
        _CACHED["nc"] = _build_nc()
    nc = _CACHED["nc"]

    in_maps = _lift_inputs(coarse_pc, fine_pc, gt_pc)
    trace = bool(os.environ.get("CHAMFER_TRACE"))
    res = run_bass_kernel_spmd(nc, in_maps, core_ids=list(range(N_CORES)), trace=trace)
    LAST_EXEC_TIME_NS = res.exec_time_ns
    results = res.results

    rowmin_c_sum = 0.0
    rowmin_f_sum = 0.0
    col_c_sum = 0.0
    col_f_sum = 0.0
    for b in range(B):
        r0 = results[2 * b]
        r1 = results[2 * b + 1]
        for r in (r0, r1):
            rm = r["out_rowmin"]
            rowmin_c_sum += rm[:, :RB_COARSE].sum(dtype=np.float64)
            rowmin_f_sum += rm[:, RB_COARSE:].sum(dtype=np.float64)
        col_c = np.minimum(r0["out_colacc_coarse"], r1["out_colacc_coarse"]).min(axis=0)
        col_f = np.minimum(r0["out_colacc_fine"], r1["out_colacc_fine"]).min(axis=0)
        col_c_sum += col_c.sum(dtype=np.float64)
        col_f_sum += col_f.sum(dtype=np.float64)

    loss_coarse = (rowmin_c_sum / (B * N_COARSE) + col_c_sum / (B * N_GT)) * float(param_coarse)
    loss_fine = (rowmin_f_sum / (B * N_FINE) + col_f_sum / (B * N_GT)) * float(param_fine)
    return np.array([loss_coarse, loss_fine], dtype=np.float32)


# revision 27
# speedup vs baseline: 1.9800x; 1.3086x over previous
"""Chamfer point-cloud completion loss on 8 Trainium2 NeuronCores.

Strategy: data-parallel over (batch, row-half). Core c handles batch c//2,
row-half c%2 of both the coarse (1024 pts) and fine (8192 pts) clouds vs the
full gt cloud (8192 pts) of that batch.

Squared L2 distances via a "lift" matmul on the PE:
  lift_x = [x0, x1, x2, |x|^2, 1]      (5 rows, M=128 x-points)
  lift_y = [-2y0, -2y1, -2y2, 1, |y|^2] (5 rows, N=512 y-points)
  d(m, n) = sum_k lift_x[k, m] * lift_y[k, n]
To get fp32-class precision at bf16 matmul speed (1 cycle/row vs 4 for
fp32), each lift row v is decomposed v = h + m + l (3x bf16) and the six
product terms hh', hm', mh', hl', lh', mm' are stacked into ONE K=30
bf16 matmul (cost is independent of K); dropped terms are O(2^-26).

The Act engine copies each PSUM group (f32) to SBUF as bf16; the DVE then
runs at 2x (16-bit dual-port): row-mins (over gt) via tensor_reduce, col-mins
(over pred) via in-place tensor_tensor min into a bf16 SBUF accumulator
colacc[128, 8192] (partition p holds min over rows r = p mod 128 of the
core's shard). bf16 rounding of the distances (~0.2% rel) is far inside the
2e-2 tolerance. Host finishes: partition-min of colacc, pair-min across the
two cores of each batch, float64 means, scale by params.
"""

import os
import sys

import numpy as np

_TRN_REPO = "/opt/trn_rl_repo"
if _TRN_REPO not in sys.path:
    sys.path.insert(0, _TRN_REPO)

B = 4
N_COARSE = 1024
N_FINE = 8192
N_GT = 8192
N_CORES = 8

ROWS_COARSE = N_COARSE // 2   # 512 rows per core
ROWS_FINE = N_FINE // 2       # 4096 rows per core
ROWS_TOTAL = ROWS_COARSE + ROWS_FINE  # 4608
RB_COARSE = ROWS_COARSE // 128  # 4 row-blocks (first)
RB_FINE = ROWS_FINE // 128      # 32 row-blocks
RB_TOTAL = RB_COARSE + RB_FINE  # 36

GROUP_W = 2048                # PSUM group width (4 banks)
N_GROUPS = N_GT // GROUP_W    # 4 groups per row-block
MM_W = 512                    # single matmul moving width
MM_PER_GROUP = GROUP_W // MM_W

LAST_EXEC_TIME_NS = None

_CACHED = {}


def _build_nc():
    import concourse.bass as bass
    import concourse.tile as tile
    from concourse import mybir
    from concourse.bacc import Bacc

    f32 = mybir.dt.float32
    bf16 = mybir.dt.bfloat16
    AX = mybir.AxisListType
    OP = mybir.AluOpType

    nc = Bacc()

    xy_lift_d = nc.dram_tensor(
        "xy_lift", [30, ROWS_TOTAL + N_GT], bf16, kind="ExternalInput"
    )
    out_rowmin_d = nc.dram_tensor("out_rowmin", [128, RB_TOTAL], f32, kind="ExternalOutput")
    out_colacc_c_d = nc.dram_tensor("out_colacc_coarse", [128, N_GT], bf16, kind="ExternalOutput")
    out_colacc_f_d = nc.dram_tensor("out_colacc_fine", [128, N_GT], bf16, kind="ExternalOutput")

    BIG = 3.0e38

    act_copy = mybir.ActivationFunctionType.Copy

    with tile.TileContext(nc) as tc:
        with (
            tc.tile_pool(name="singles", bufs=1) as singles,
            tc.tile_pool(name="work", bufs=3) as work,
            tc.tile_pool(name="copies", bufs=3) as copies,
            tc.tile_pool(name="psum", bufs=2, space="PSUM") as psum_pool,
        ):
            xy_lift = singles.tile([30, ROWS_TOTAL + N_GT], bf16)
            nc.sync.dma_start(out=xy_lift[:], in_=xy_lift_d[:])
            x_lift = xy_lift[:, :ROWS_TOTAL]
            y_lift = xy_lift[:, ROWS_TOTAL:]

            colacc_c = singles.tile([128, N_GT], bf16)
            colacc_f = singles.tile([128, N_GT], bf16)
            rowmin_all = singles.tile([128, RB_TOTAL], f32)

            nc.vector.memset(colacc_c[:], BIG)
            nc.vector.memset(colacc_f[:], BIG)

            for rb in range(RB_TOTAL):
                colacc = colacc_c if rb < RB_COARSE else colacc_f
                rowmin_tmp = work.tile([128, N_GROUPS], f32, name="rowmin_tmp")
                for g in range(N_GROUPS):
                    pg = psum_pool.tile([128, GROUP_W], f32, name="pg")
                    for k in range(MM_PER_GROUP):
                        c0 = g * GROUP_W + k * MM_W
                        nc.tensor.matmul(
                            pg[:, k * MM_W:(k + 1) * MM_W],
                            x_lift[:, rb * 128:(rb + 1) * 128],
                            y_lift[:, c0:c0 + MM_W],
                        )
                    cp = copies.tile([128, GROUP_W], bf16, name="cp")
                    nc.scalar.activation(out=cp[:], in_=pg[:], func=act_copy)
                    nc.vector.tensor_reduce(
                        out=rowmin_tmp[:, g:g + 1], in_=cp[:], axis=AX.X, op=OP.min,
                    )
                    acc = colacc[:, g * GROUP_W:(g + 1) * GROUP_W]
                    nc.vector.tensor_tensor(
                        out=acc, in0=cp[:], in1=acc, op=OP.min,
                    )
                nc.vector.tensor_reduce(
                    out=rowmin_all[:, rb:rb + 1], in_=rowmin_tmp[:], axis=AX.X, op=OP.min,
                )

            nc.sync.dma_start(out=out_rowmin_d[:], in_=rowmin_all[:])
            nc.sync.dma_start(out=out_colacc_c_d[:], in_=colacc_c[:])
            nc.sync.dma_start(out=out_colacc_f_d[:], in_=colacc_f[:])

    nc.finalize()
    return nc


def _bf16_split3(v):
    """v (f32/f64) -> (h, m, l) bf16 arrays with h+m+l ~= v to ~2^-26."""
    import ml_dtypes

    bf = ml_dtypes.bfloat16
    v = v.astype(np.float64)
    h = v.astype(bf)
    r = v - h.astype(np.float64)
    m = r.astype(bf)
    l = (r - m.astype(np.float64)).astype(bf)
    return h, m, l


def _lift_inputs(coarse_pc, fine_pc, gt_pc):
    """Per-core lifted input arrays (bf16 triple-decomposed, K=30)."""
    import ml_dtypes

    bf = ml_dtypes.bfloat16
    in_maps = []
    for c in range(N_CORES):
        b, h = divmod(c, 2)
        C = coarse_pc[b, h * ROWS_COARSE:(h + 1) * ROWS_COARSE]  # [512, 3]
        F = fine_pc[b, h * ROWS_FINE:(h + 1) * ROWS_FINE]        # [4096, 3]
        X = np.concatenate([C, F], axis=0).astype(np.float64)    # [4608, 3]
        Y = gt_pc[b].astype(np.float64)                          # [8192, 3]

        lift_x = np.empty((5, ROWS_TOTAL), dtype=np.float64)
        lift_x[0:3] = X.T
        lift_x[3] = (X * X).sum(axis=1)
        lift_x[4] = 1.0
        lift_y = np.empty((5, N_GT), dtype=np.float64)
        lift_y[0:3] = -2.0 * Y.T
        lift_y[3] = 1.0
        lift_y[4] = (Y * Y).sum(axis=1)

        xh, xm, xl = _bf16_split3(lift_x)
        yh, ym, yl = _bf16_split3(lift_y)

        # sum over the six stacked blocks = hh' + hm' + mh' + hl' + lh' + mm'
        x_blocks = (xh, xh, xm, xh, xl, xm)
        y_blocks = (yh, ym, yh, yl, yh, ym)
        xy_lift = np.empty((30, ROWS_TOTAL + N_GT), dtype=bf)
        for i in range(6):
            xy_lift[5 * i:5 * i + 5, :ROWS_TOTAL] = x_blocks[i]
            xy_lift[5 * i:5 * i + 5, ROWS_TOTAL:] = y_blocks[i]

        in_maps.append({"xy_lift": xy_lift})
    return in_maps


def kernel(coarse_pc, fine_pc, gt_pc, param_coarse, param_fine):
    global LAST_EXEC_TIME_NS
    from concourse.bass_utils import run_bass_kernel_spmd

    coarse_pc = np.asarray(coarse_pc, dtype=np.float32)
    fine_pc = np.asarray(fine_pc, dtype=np.float32)
    gt_pc = np.asarray(gt_pc, dtype=np.float32)

    if "nc" not in _CACHED:
        _CACHED["nc"] = _build_nc()
    nc = _CACHED["nc"]

    in_maps = _lift_inputs(coarse_pc, fine_pc, gt_pc)
    trace = bool(os.environ.get("CHAMFER_TRACE"))
    res = run_bass_kernel_spmd(nc, in_maps, core_ids=list(range(N_CORES)), trace=trace)
    LAST_EXEC_TIME_NS = res.exec_time_ns
    results = res.results

    rowmin_c_sum = 0.0
    rowmin_f_sum = 0.0
    col_c_sum = 0.0
    col_f_sum = 0.0
    for b in range(B):
        r0 = results[2 * b]
        r1 = results[2 * b + 1]
        for r in (r0, r1):
            rm = r["out_rowmin"]
            rowmin_c_sum += rm[:, :RB_COARSE].sum(dtype=np.float64)
            rowmin_f_sum += rm[:, RB_COARSE:].sum(dtype=np.float64)
        cc0 = r0["out_colacc_coarse"].astype(np.float32)
        cc1 = r1["out_colacc_coarse"].astype(np.float32)
        cf0 = r0["out_colacc_fine"].astype(np.float32)
        cf1 = r1["out_colacc_fine"].astype(np.float32)
        col_c = np.minimum(cc0, cc1).min(axis=0)
        col_f = np.minimum(cf0, cf1).min(axis=0)
        col_c_sum += col_c.sum(dtype=np.float64)
        col_f_sum += col_f.sum(dtype=np.float64)

    loss_coarse = (rowmin_c_sum / (B * N_COARSE) + col_c_sum / (B * N_GT)) * float(param_coarse)
    loss_fine = (rowmin_f_sum / (B * N_FINE) + col_f_sum / (B * N_GT)) * float(param_fine)
    return np.array([loss_coarse, loss_fine], dtype=np.float32)


# revision 30
# speedup vs baseline: 2.3058x; 1.1646x over previous
"""Chamfer point-cloud completion loss on 8 Trainium2 NeuronCores.

Strategy: data-parallel over (batch, row-half). Core c handles batch c//2,
row-half c%2 of both the coarse (1024 pts) and fine (8192 pts) clouds vs the
full gt cloud (8192 pts) of that batch.

Squared L2 distances via a "lift" matmul on the PE:
  lift_x = [x0, x1, x2, |x|^2, 1]      (5 rows, M=128 x-points)
  lift_y = [-2y0, -2y1, -2y2, 1, |y|^2] (5 rows, N=512 y-points)
  d(m, n) = sum_k lift_x[k, m] * lift_y[k, n]
To get fp32-class precision at bf16 matmul speed (1 cycle/row vs 4 for
fp32), each lift row v is decomposed v = h + m + l (3x bf16) and the six
product terms hh', hm', mh', hl', lh', mm' are stacked into ONE K=30
bf16 matmul (cost is independent of K); dropped terms are O(2^-26).

The Act engine copies each PSUM group (f32) to SBUF as bf16; the DVE then
runs almost entirely at 2x (16-bit dual-port tensor_tensor): col-mins (over
pred) via in-place TT min into a bf16 SBUF accumulator colacc[128, 8192]
(partition p holds min over rows r = p mod 128 of the core's shard);
row-mins (over gt) via TT min accumulation across the 4 groups of a
row-block into the group-0 copy buffer, finished by a TT halving tree +
one narrow tensor_reduce (full-width tensor_reduce only runs at 1x even
for bf16 — measured). bf16 rounding of the distances (~0.2% rel) is far
inside the 2e-2 tolerance. Host finishes: partition-min of colacc,
pair-min across the two cores of each batch, float64 means, scale by
params.
"""

import os
import sys

import numpy as np

_TRN_REPO = "/opt/trn_rl_repo"
if _TRN_REPO not in sys.path:
    sys.path.insert(0, _TRN_REPO)

B = 4
N_COARSE = 1024
N_FINE = 8192
N_GT = 8192
N_CORES = 8

ROWS_COARSE = N_COARSE // 2   # 512 rows per core
ROWS_FINE = N_FINE // 2       # 4096 rows per core
ROWS_TOTAL = ROWS_COARSE + ROWS_FINE  # 4608
RB_COARSE = ROWS_COARSE // 128  # 4 row-blocks (first)
RB_FINE = ROWS_FINE // 128      # 32 row-blocks
RB_TOTAL = RB_COARSE + RB_FINE  # 36

GROUP_W = 2048                # PSUM group width (4 banks)
N_GROUPS = N_GT // GROUP_W    # 4 groups per row-block
MM_W = 512                    # single matmul moving width
MM_PER_GROUP = GROUP_W // MM_W

LAST_EXEC_TIME_NS = None

_CACHED = {}


def _build_nc():
    import concourse.bass as bass
    import concourse.tile as tile
    from concourse import mybir
    from concourse.bacc import Bacc

    f32 = mybir.dt.float32
    bf16 = mybir.dt.bfloat16
    AX = mybir.AxisListType
    OP = mybir.AluOpType

    nc = Bacc()

    xy_lift_d = nc.dram_tensor(
        "xy_lift", [30, ROWS_TOTAL + N_GT], bf16, kind="ExternalInput"
    )
    out_rowmin_d = nc.dram_tensor("out_rowmin", [128, RB_TOTAL], f32, kind="ExternalOutput")
    out_colacc_c_d = nc.dram_tensor("out_colacc_coarse", [128, N_GT], bf16, kind="ExternalOutput")
    out_colacc_f_d = nc.dram_tensor("out_colacc_fine", [128, N_GT], bf16, kind="ExternalOutput")

    BIG = 3.0e38

    act_copy = mybir.ActivationFunctionType.Copy

    with tile.TileContext(nc) as tc:
        with (
            tc.tile_pool(name="singles", bufs=1) as singles,
            tc.tile_pool(name="copies", bufs=6) as copies,
            tc.tile_pool(name="psum", bufs=2, space="PSUM") as psum_pool,
        ):
            xy_lift = singles.tile([30, ROWS_TOTAL + N_GT], bf16)
            nc.sync.dma_start(out=xy_lift[:], in_=xy_lift_d[:])
            x_lift = xy_lift[:, :ROWS_TOTAL]
            y_lift = xy_lift[:, ROWS_TOTAL:]

            colacc_c = singles.tile([128, N_GT], bf16)
            colacc_f = singles.tile([128, N_GT], bf16)
            rowmin_all = singles.tile([128, RB_TOTAL], f32)

            nc.vector.memset(colacc_c[:], BIG)
            nc.vector.memset(colacc_f[:], BIG)

            for rb in range(RB_TOTAL):
                colacc = colacc_c if rb < RB_COARSE else colacc_f
                rowacc = None
                for g in range(N_GROUPS):
                    pg = psum_pool.tile([128, GROUP_W], f32, name="pg")
                    for k in range(MM_PER_GROUP):
                        c0 = g * GROUP_W + k * MM_W
                        nc.tensor.matmul(
                            pg[:, k * MM_W:(k + 1) * MM_W],
                            x_lift[:, rb * 128:(rb + 1) * 128],
                            y_lift[:, c0:c0 + MM_W],
                        )
                    cp = copies.tile([128, GROUP_W], bf16, name="cp")
                    nc.scalar.activation(out=cp[:], in_=pg[:], func=act_copy)
                    acc = colacc[:, g * GROUP_W:(g + 1) * GROUP_W]
                    nc.vector.tensor_tensor(
                        out=acc, in0=cp[:], in1=acc, op=OP.min,
                    )
                    if g == 0:
                        rowacc = cp
                    else:
                        nc.vector.tensor_tensor(
                            out=rowacc[:], in0=cp[:], in1=rowacc[:], op=OP.min,
                        )
                w = GROUP_W
                while w > 256:
                    h = w // 2
                    nc.vector.tensor_tensor(
                        out=rowacc[:, :h], in0=rowacc[:, h:w], in1=rowacc[:, :h],
                        op=OP.min,
                    )
                    w = h
                nc.vector.tensor_reduce(
                    out=rowmin_all[:, rb:rb + 1], in_=rowacc[:, :w], axis=AX.X, op=OP.min,
                )

            nc.sync.dma_start(out=out_rowmin_d[:], in_=rowmin_all[:])
            nc.sync.dma_start(out=out_colacc_c_d[:], in_=colacc_c[:])
            nc.sync.dma_start(out=out_colacc_f_d[:], in_=colacc_f[:])

    nc.finalize()
    return nc


def _bf16_split3(v):
    """v (f32/f64) -> (h, m, l) bf16 arrays with h+m+l ~= v to ~2^-26."""
    import ml_dtypes

    bf = ml_dtypes.bfloat16
    v = v.astype(np.float64)
    h = v.astype(bf)
    r = v - h.astype(np.float64)
    m = r.astype(bf)
    l = (r - m.astype(np.float64)).astype(bf)
    return h, m, l


def _lift_inputs(coarse_pc, fine_pc, gt_pc):
    """Per-core lifted input arrays (bf16 triple-decomposed, K=30)."""
    import ml_dtypes

    bf = ml_dtypes.bfloat16
    in_maps = []
    for c in range(N_CORES):
        b, h = divmod(c, 2)
        C = coarse_pc[b, h * ROWS_COARSE:(h + 1) * ROWS_COARSE]  # [512, 3]
        F = fine_pc[b, h * ROWS_FINE:(h + 1) * ROWS_FINE]        # [4096, 3]
        X = np.concatenate([C, F], axis=0).astype(np.float64)    # [4608, 3]
        Y = gt_pc[b].astype(np.float64)                          # [8192, 3]

        lift_x = np.empty((5, ROWS_TOTAL), dtype=np.float64)
        lift_x[0:3] = X.T
        lift_x[3] = (X * X).sum(axis=1)
        lift_x[4] = 1.0
        lift_y = np.empty((5, N_GT), dtype=np.float64)
        lift_y[0:3] = -2.0 * Y.T
        lift_y[3] = 1.0
        lift_y[4] = (Y * Y).sum(axis=1)

        xh, xm, xl = _bf16_split3(lift_x)
        yh, ym, yl = _bf16_split3(lift_y)

        # sum over the six stacked blocks = hh' + hm' + mh' + hl' + lh' + mm'
        x_blocks = (xh, xh, xm, xh, xl, xm)
        y_blocks = (yh, ym, yh, yl, yh, ym)
        xy_lift = np.empty((30, ROWS_TOTAL + N_GT), dtype=bf)
        for i in range(6):
            xy_lift[5 * i:5 * i + 5, :ROWS_TOTAL] = x_blocks[i]
            xy_lift[5 * i:5 * i + 5, ROWS_TOTAL:] = y_blocks[i]

        in_maps.append({"xy_lift": xy_lift})
    return in_maps


def kernel(coarse_pc, fine_pc, gt_pc, param_coarse, param_fine):
    global LAST_EXEC_TIME_NS
    from concourse.bass_utils import run_bass_kernel_spmd

    coarse_pc = np.asarray(coarse_pc, dtype=np.float32)
    fine_pc = np.asarray(fine_pc, dtype=np.float32)
    gt_pc = np.asarray(gt_pc, dtype=np.float32)

    if "nc" not in _CACHED:
        _CACHED["nc"] = _build_nc()
    nc = _CACHED["nc"]

    in_maps = _lift_inputs(coarse_pc, fine_pc, gt_pc)
    trace = bool(os.environ.get("CHAMFER_TRACE"))
    res = run_bass_kernel_spmd(nc, in_maps, core_ids=list(range(N_CORES)), trace=trace)
    LAST_EXEC_TIME_NS = res.exec_time_ns
    results = res.results

    rowmin_c_sum = 0.0
    rowmin_f_sum = 0.0
    col_c_sum = 0.0
    col_f_sum = 0.0
    for b in range(B):
        r0 = results[2 * b]
        r1 = results[2 * b + 1]
        for r in (r0, r1):
            rm = r["out_rowmin"]
            rowmin_c_sum += rm[:, :RB_COARSE].sum(dtype=np.float64)
            rowmin_f_sum += rm[:, RB_COARSE:].sum(dtype=np.float64)
        cc0 = r0["out_colacc_coarse"].astype(np.float32)
        cc1 = r1["out_colacc_coarse"].astype(np.float32)
        cf0 = r0["out_colacc_fine"].astype(np.float32)
        cf1 = r1["out_colacc_fine"].astype(np.float32)
        col_c = np.minimum(cc0, cc1).min(axis=0)
        col_f = np.minimum(cf0, cf1).min(axis=0)
        col_c_sum += col_c.sum(dtype=np.float64)
        col_f_sum += col_f.sum(dtype=np.float64)

    loss_coarse = (rowmin_c_sum / (B * N_COARSE) + col_c_sum / (B * N_GT)) * float(param_coarse)
    loss_fine = (rowmin_f_sum / (B * N_FINE) + col_f_sum / (B * N_GT)) * float(param_fine)
    return np.array([loss_coarse, loss_fine], dtype=np.float32)


# revision 32
# speedup vs baseline: 2.7291x; 1.1836x over previous
"""Chamfer point-cloud completion loss on 8 Trainium2 NeuronCores.

Strategy: data-parallel over (batch, row-half). Core c handles batch c//2,
row-half c%2 of both the coarse (1024 pts) and fine (8192 pts) clouds vs the
full gt cloud (8192 pts) of that batch.

Squared L2 distances via a "lift" matmul on the PE:
  lift_x = [x0, x1, x2, |x|^2, 1]      (5 rows, M=128 x-points)
  lift_y = [-2y0, -2y1, -2y2, 1, |y|^2] (5 rows, N=512 y-points)
  d(m, n) = sum_k lift_x[k, m] * lift_y[k, n]
To get fp32-class precision at bf16 matmul speed (1 cycle/row vs 4 for
fp32), each lift row v is decomposed v = h + m + l (3x bf16) and the six
product terms hh', hm', mh', hl', lh', mm' are stacked into ONE K=30
bf16 matmul (cost is independent of K); dropped terms are O(2^-26).

The Act engine copies each PSUM group (f32) to SBUF as bf16; the DVE then
runs almost entirely at 2x (16-bit dual-port tensor_tensor): col-mins (over
pred) via in-place TT min into a bf16 SBUF accumulator colacc[128, 8192]
(partition p holds min over rows r = p mod 128 of the core's shard);
row-mins (over gt) via TT min accumulation across the 4 groups of a
row-block into the group-0 copy buffer, finished by a TT halving tree +
one narrow tensor_reduce (full-width tensor_reduce only runs at 1x even
for bf16 — measured). bf16 rounding of the distances (~0.2% rel) is far
inside the 2e-2 tolerance. Host finishes: partition-min of colacc,
pair-min across the two cores of each batch, float64 means, scale by
params.
"""

import os
import sys

import numpy as np

_TRN_REPO = "/opt/trn_rl_repo"
if _TRN_REPO not in sys.path:
    sys.path.insert(0, _TRN_REPO)

B = 4
N_COARSE = 1024
N_FINE = 8192
N_GT = 8192
N_CORES = 8

ROWS_COARSE = N_COARSE // 2   # 512 rows per core
ROWS_FINE = N_FINE // 2       # 4096 rows per core
ROWS_TOTAL = ROWS_COARSE + ROWS_FINE  # 4608
RB_COARSE = ROWS_COARSE // 128  # 4 row-blocks (first)
RB_FINE = ROWS_FINE // 128      # 32 row-blocks
RB_TOTAL = RB_COARSE + RB_FINE  # 36

GROUP_W = 2048                # PSUM group width (4 banks)
N_GROUPS = N_GT // GROUP_W    # 4 groups per row-block
MM_W = 512                    # single matmul moving width
MM_PER_GROUP = GROUP_W // MM_W

LAST_EXEC_TIME_NS = None

_CACHED = {}


def _build_nc():
    import concourse.bass as bass
    import concourse.tile as tile
    from concourse import mybir
    from concourse.bacc import Bacc

    f32 = mybir.dt.float32
    bf16 = mybir.dt.bfloat16
    AX = mybir.AxisListType
    OP = mybir.AluOpType

    nc = Bacc()

    xy_lift_d = nc.dram_tensor(
        "xy_lift", [30, ROWS_TOTAL + N_GT], bf16, kind="ExternalInput"
    )
    out_rowmin_d = nc.dram_tensor("out_rowmin", [128, RB_TOTAL], f32, kind="ExternalOutput")
    out_colacc_c_d = nc.dram_tensor("out_colacc_coarse", [128, N_GT], bf16, kind="ExternalOutput")
    out_colacc_f_d = nc.dram_tensor("out_colacc_fine", [128, N_GT], bf16, kind="ExternalOutput")

    BIG = 3.0e38

    act_copy = mybir.ActivationFunctionType.Copy

    with tile.TileContext(nc) as tc:
        with (
            tc.tile_pool(name="singles", bufs=1) as singles,
            tc.tile_pool(name="copies", bufs=6) as copies,
            tc.tile_pool(name="psum", bufs=2, space="PSUM") as psum_pool,
        ):
            xy_lift = singles.tile([30, ROWS_TOTAL + N_GT], bf16)
            nc.sync.dma_start(out=xy_lift[:], in_=xy_lift_d[:])
            x_lift = xy_lift[:, :ROWS_TOTAL]
            y_lift = xy_lift[:, ROWS_TOTAL:]

            colacc_c = singles.tile([128, N_GT], bf16)
            colacc_f = singles.tile([128, N_GT], bf16)
            rowmin_all = singles.tile([128, RB_TOTAL], f32)

            nc.gpsimd.memset(colacc_c[:], BIG)
            nc.gpsimd.memset(colacc_f[:], BIG)

            for rb in range(RB_TOTAL):
                colacc = colacc_c if rb < RB_COARSE else colacc_f
                rowacc = None
                for g in range(N_GROUPS):
                    pg = psum_pool.tile([128, GROUP_W], f32, name="pg")
                    for k in range(MM_PER_GROUP):
                        c0 = g * GROUP_W + k * MM_W
                        nc.tensor.matmul(
                            pg[:, k * MM_W:(k + 1) * MM_W],
                            x_lift[:, rb * 128:(rb + 1) * 128],
                            y_lift[:, c0:c0 + MM_W],
                        )
                    cp = copies.tile([128, GROUP_W], bf16, name="cp")
                    nc.scalar.activation(out=cp[:], in_=pg[:], func=act_copy)
                    acc = colacc[:, g * GROUP_W:(g + 1) * GROUP_W]
                    nc.vector.tensor_tensor(
                        out=acc, in0=cp[:], in1=acc, op=OP.min,
                    )
                    if g == 0:
                        rowacc = cp
                    else:
                        nc.vector.tensor_tensor(
                            out=rowacc[:], in0=cp[:], in1=rowacc[:], op=OP.min,
                        )
                h = GROUP_W // 2
                nc.vector.tensor_tensor(
                    out=rowacc[:, :h], in0=rowacc[:, h:], in1=rowacc[:, :h],
                    op=OP.min,
                )
                nc.vector.tensor_reduce(
                    out=rowmin_all[:, rb:rb + 1], in_=rowacc[:, :h], axis=AX.X, op=OP.min,
                )

            nc.sync.dma_start(out=out_rowmin_d[:], in_=rowmin_all[:])
            nc.sync.dma_start(out=out_colacc_c_d[:], in_=colacc_c[:])
            nc.sync.dma_start(out=out_colacc_f_d[:], in_=colacc_f[:])

    nc.finalize()
    return nc


def _bf16_split3(v):
    """v (f32/f64) -> (h, m, l) bf16 arrays with h+m+l ~= v to ~2^-26."""
    import ml_dtypes

    bf = ml_dtypes.bfloat16
    v = v.astype(np.float64)
    h = v.astype(bf)
    r = v - h.astype(np.float64)
    m = r.astype(bf)
    l = (r - m.astype(np.float64)).astype(bf)
    return h, m, l


def _lift_inputs(coarse_pc, fine_pc, gt_pc):
    """Per-core lifted input arrays (bf16 triple-decomposed, K=30)."""
    import ml_dtypes

    bf = ml_dtypes.bfloat16
    in_maps = []
    for c in range(N_CORES):
        b, h = divmod(c, 2)
        C = coarse_pc[b, h * ROWS_COARSE:(h + 1) * ROWS_COARSE]  # [512, 3]
        F = fine_pc[b, h * ROWS_FINE:(h + 1) * ROWS_FINE]        # [4096, 3]
        X = np.concatenate([C, F], axis=0).astype(np.float64)    # [4608, 3]
        Y = gt_pc[b].astype(np.float64)                          # [8192, 3]

        lift_x = np.empty((5, ROWS_TOTAL), dtype=np.float64)
        lift_x[0:3] = X.T
        lift_x[3] = (X * X).sum(axis=1)
        lift_x[4] = 1.0
        lift_y = np.empty((5, N_GT), dtype=np.float64)
        lift_y[0:3] = -2.0 * Y.T
        lift_y[3] = 1.0
        lift_y[4] = (Y * Y).sum(axis=1)

        xh, xm, xl = _bf16_split3(lift_x)
        yh, ym, yl = _bf16_split3(lift_y)

        # sum over the six stacked blocks = hh' + hm' + mh' + hl' + lh' + mm'
        x_blocks = (xh, xh, xm, xh, xl, xm)
        y_blocks = (yh, ym, yh, yl, yh, ym)
        xy_lift = np.empty((30, ROWS_TOTAL + N_GT), dtype=bf)
        for i in range(6):
            xy_lift[5 * i:5 * i + 5, :ROWS_TOTAL] = x_blocks[i]
            xy_lift[5 * i:5 * i + 5, ROWS_TOTAL:] = y_blocks[i]

        in_maps.append({"xy_lift": xy_lift})
    return in_maps


def kernel(coarse_pc, fine_pc, gt_pc, param_coarse, param_fine):
    global LAST_EXEC_TIME_NS
    from concourse.bass_utils import run_bass_kernel_spmd

    coarse_pc = np.asarray(coarse_pc, dtype=np.float32)
    fine_pc = np.asarray(fine_pc, dtype=np.float32)
    gt_pc = np.asarray(gt_pc, dtype=np.float32)

    if "nc" not in _CACHED:
        _CACHED["nc"] = _build_nc()
    nc = _CACHED["nc"]

    in_maps = _lift_inputs(coarse_pc, fine_pc, gt_pc)
    trace = bool(os.environ.get("CHAMFER_TRACE"))
    res = run_bass_kernel_spmd(nc, in_maps, core_ids=list(range(N_CORES)), trace=trace)
    LAST_EXEC_TIME_NS = res.exec_time_ns
    results = res.results

    rowmin_c_sum = 0.0
    rowmin_f_sum = 0.0
    col_c_sum = 0.0
    col_f_sum = 0.0
    for b in range(B):
        r0 = results[2 * b]
        r1 = results[2 * b + 1]
        for r in (r0, r1):
            rm = r["out_rowmin"]
            rowmin_c_sum += rm[:, :RB_COARSE].sum(dtype=np.float64)
            rowmin_f_sum += rm[:, RB_COARSE:].sum(dtype=np.float64)
        cc0 = r0["out_colacc_coarse"].astype(np.float32)
        cc1 = r1["out_colacc_coarse"].astype(np.float32)
        cf0 = r0["out_colacc_fine"].astype(np.float32)
        cf1 = r1["out_colacc_fine"].astype(np.float32)
        col_c = np.minimum(cc0, cc1).min(axis=0)
        col_f = np.minimum(cf0, cf1).min(axis=0)
        col_c_sum += col_c.sum(dtype=np.float64)
        col_f_sum += col_f.sum(dtype=np.float64)

    loss_coarse = (rowmin_c_sum / (B * N_COARSE) + col_c_sum / (B * N_GT)) * float(param_coarse)
    loss_fine = (rowmin_f_sum / (B * N_FINE) + col_f_sum / (B * N_GT)) * float(param_fine)
    return np.array([loss_coarse, loss_fine], dtype=np.float32)
